# revision 7
# baseline (speedup 1.0000x reference)
"""DTIGCN message-passing kernel for 8 Trainium2 NeuronCores.

Strategy (per spec sharding hint): shard every adjacency matrix row-wise
(destination dim) across the 8 cores; replicate the [64,64] weights and
source features. Each core computes rn(A_shard) @ act(X @ W^T + b) for
its rows plus the local mean+l2norm — no cross-core reduction needed.

Key layout choices (host-side prep, done inside kernel()):
  - A shards are shipped TRANSPOSED ([S, R_shard], fp16) so the device
    loads tiles with the contraction (source) dim on SBUF partitions at
    full DMA bandwidth — no on-chip transposes.
  - Features are shipped transposed with a ones row appended ([65, S]),
    and weights as [W^T; b] packs, so H = act(X@W^T + b) is one matmul
    per 128-row block per source type.
  - H gets a ones column appended: the main matmul A_shard @ [H | 1]
    yields both A@H and the row sums (for row normalization) in one pass.
  - elu is computed as max(z,0) + exp(min(z,0))  (i.e. elu + 1); the -1
    is recovered for free because rn(A) rows sum to exactly 1, by
    subtracting the count of elu-messages per destination at the end.
  - mean folds into the l2 norm: l2norm(s/n) = s / max(||s||, n*eps).
"""

import os
import numpy as np

ND, NPR, NDI, NSE, D = 4000, 4000, 3000, 2000, 64
N_CORES = 8

# relations of each source type; relu relation first, then elu relations
SRC_PACKS = {
    "drug": ["dd", "pd", "did", "sed"],
    "protein": ["dp", "pp", "dip"],
    "disease": ["ddi", "pdi"],
    "sideeffect": ["dse"],
}
REL_DEST = {
    "dd": "drug", "dp": "drug", "ddi": "drug", "dse": "drug",
    "pp": "protein", "pd": "protein", "pdi": "protein",
    "did": "disease", "dip": "disease", "sed": "sideeffect",
}
REL_RELU = {r: r in ("dd", "dp", "ddi", "dse") for r in REL_DEST}
TYPE_N = {"drug": ND, "protein": NPR, "disease": NDI, "sideeffect": NSE}
TYPES = ["drug", "protein", "disease", "sideeffect"]
FEAT_KEY = {t: f"feat_{t}" for t in TYPES}
N_MEAN = {"drug": 5, "protein": 4, "disease": 3, "sideeffect": 2}
N_ELU_DEST = {"drug": 0, "protein": 3, "disease": 2, "sideeffect": 1}
MAIN_ORDER = ["dd", "dp", "ddi", "dse", "pp", "pd", "pdi", "did", "dip", "sed"]
REL_SRC = {r: t for t, rels in SRC_PACKS.items() for r in rels}
G_BLOCKS = 8  # 128-row source blocks per AT dma batch


def _ceil_div(a, b):
    return (a + b - 1) // b


def _split_sync_waits(nc, max_waits=1):
    """Hoist excess per-instruction sem waits onto preceding NOPs.

    The staged walrus build rejects >max_waits sync waits on a single
    instruction ("Too many sync wait commands" in
    CoreV3GenImpl::setupSyncWait). Each engine executes its stream in
    order, so satisfying the extra waits on same-engine NOPs emitted
    immediately before the instruction is semantically identical.
    """
    import bass_rust

    cur_bb = nc.cur_bb
    assert cur_bb is not None
    host_list = cur_bb.bb.instructions

    def make_nop(engine, wait):
        bi = nc.engines[engine].nop()
        # un-append it from the current bb; we place it manually below
        assert host_list and host_list[-1] is bi.ins
        host_list.pop()
        bi.ins.sync_info = bass_rust.SyncInfo(on_wait=[wait], on_update=[])
        return bi.ins

    for f in nc.m.functions:
        for bb in f.blocks:
            new_insts = []
            changed = False
            for inst in bb.instructions:
                si = inst.sync_info
                waits = list(si.on_wait) if si and si.on_wait else []
                if len(waits) > max_waits:
                    for w in waits[max_waits:]:
                        new_insts.append(make_nop(inst.engine, w))
                    inst.sync_info = bass_rust.SyncInfo(
                        on_wait=waits[:max_waits],
                        on_update=list(si.on_update) if si.on_update else [],
                    )
                    changed = True
                new_insts.append(inst)
            if changed:
                bb.instructions = new_insts


def build_nc():
    """Build the per-core Bass program (identical across the 8 cores)."""
    from contextlib import ExitStack

    import concourse.bass as bass
    import concourse.mybir as mybir
    import concourse.tile as tile

    f32, f16 = mybir.dt.float32, mybir.dt.float16
    AF = mybir.ActivationFunctionType
    OP = mybir.AluOpType

    nc = bass.Bass()
    at_dram = {}
    for r in MAIN_ORDER:
        S = TYPE_N[REL_SRC[r]]
        R = TYPE_N[REL_DEST[r]] // N_CORES
        at_dram[r] = nc.dram_tensor(f"at_{r}", [S, R], f16, kind="ExternalInput")
    xaugt_dram = {
        t: nc.dram_tensor(f"xaugt_{t}", [D + 1, TYPE_N[t]], f32, kind="ExternalInput")
        for t in TYPES
    }
    wpack_dram = {
        t: nc.dram_tensor(
            f"wpack_{t}", [D + 1, D * len(SRC_PACKS[t])], f32, kind="ExternalInput"
        )
        for t in TYPES
    }
    featd_dram = {
        t: nc.dram_tensor(
            f"featd_{t}", [TYPE_N[t] // N_CORES, D], f32, kind="ExternalInput"
        )
        for t in TYPES
    }
    out_dram = {
        t: nc.dram_tensor(
            f"out_{t}", [TYPE_N[t] // N_CORES, D], f32, kind="ExternalOutput"
        )
        for t in TYPES
    }

    with tile.TileContext(nc) as tc, ExitStack() as ctx:
        persist = ctx.enter_context(tc.tile_pool(name="persist", bufs=1))
        atpool = ctx.enter_context(tc.tile_pool(name="atpool", bufs=4))
        etmp = ctx.enter_context(tc.tile_pool(name="etmp", bufs=3))
        outp = ctx.enter_context(tc.tile_pool(name="outp", bufs=4))
        hpsum = ctx.enter_context(tc.tile_pool(name="hpsum", bufs=2, space="PSUM"))
        opsum = ctx.enter_context(tc.tile_pool(name="opsum", bufs=6, space="PSUM"))

        # ---- persistent loads -------------------------------------------
        xt_sb, wp_sb, haug, acc = {}, {}, {}, {}
        for t in TYPES:
            S, k = TYPE_N[t], len(SRC_PACKS[t])
            nblk = _ceil_div(S, 128)
            xt = persist.tile([D + 1, S], f32, name=f"xt_{t}")
            nc.scalar.dma_start(out=xt, in_=xaugt_dram[t][:, :])
            xt_sb[t] = xt
            wp = persist.tile([D + 1, D * k], f32, name=f"wp_{t}")
            nc.scalar.dma_start(out=wp, in_=wpack_dram[t][:, :])
            wp_sb[t] = wp
            hg = persist.tile([128, nblk, k, D + 1], f16, name=f"haug_{t}")
            nc.vector.memset(hg[:, :, :, D:D + 1], 1.0)  # ones column
            haug[t] = hg

            R = TYPE_N[t] // N_CORES
            nrc = _ceil_div(R, 128)
            ac = persist.tile([128, nrc, D], f32, name=f"acc_{t}")
            for rc in range(nrc):
                rcw = min(128, R - rc * 128)
                nc.scalar.dma_start(
                    out=ac[0:rcw, rc, :],
                    in_=featd_dram[t][rc * 128 : rc * 128 + rcw, :],
                )
            acc[t] = ac

        # ---- H phase: H_aug = [act(X@W^T + b) | 1] per source type ------
        for t in TYPES:
            S, k = TYPE_N[t], len(SRC_PACKS[t])
            nblk = _ceil_div(S, 128)
            ne = k - 1  # number of elu relations in this pack
            for b in range(nblk):
                kb = min(128, S - b * 128)
                ph = hpsum.tile([128, D * k], f32, name="ph", tag="ph")
                nc.tensor.matmul(
                    ph[0:kb, :],
                    lhsT=xt_sb[t][:, b * 128 : b * 128 + kb],
                    rhs=wp_sb[t][:, :],
                    start=True,
                    stop=True,
                )
                nc.scalar.activation(
                    haug[t][0:kb, b, 0, 0:D], ph[0:kb, 0:D], AF.Relu
                )
                if ne:
                    w = D * ne
                    zmin = etmp.tile([128, D * 3], f32, name="zmin", tag="zmin")
                    zexp = etmp.tile([128, D * 3], f32, name="zexp", tag="zexp")
                    zmax = etmp.tile([128, D * 3], f32, name="zmax", tag="zmax")
                    nc.vector.tensor_scalar(
                        zmin[0:kb, 0:w], ph[0:kb, D : D + w], 0.0, None, OP.min
                    )
                    nc.scalar.activation(zexp[0:kb, 0:w], zmin[0:kb, 0:w], AF.Exp)
                    nc.vector.tensor_scalar(
                        zmax[0:kb, 0:w], ph[0:kb, D : D + w], 0.0, None, OP.max
                    )
                    nc.vector.tensor_tensor(
                        out=haug[t][0:kb, b, 1:k, 0:D],
                        in0=zmax[0:kb, 0:w].rearrange("p (j f) -> p j f", f=D),
                        in1=zexp[0:kb, 0:w].rearrange("p (j f) -> p j f", f=D),
                        op=OP.add,
                    )

        # ---- main phase: per relation, O = A_shard @ H_aug --------------
        for r in MAIN_ORDER:
            t, dst = REL_SRC[r], REL_DEST[r]
            j = SRC_PACKS[t].index(r)
            S = TYPE_N[t]
            R = TYPE_N[dst] // N_CORES
            nrc = _ceil_div(R, 128)
            nblk = _ceil_div(S, 128)
            nfull, tail = S // 128, S % 128

            po = [
                opsum.tile([128, D + 1], f32, name=f"po_{r}_{rc}", tag="po")
                for rc in range(nrc)
            ]

            def mm_block(src_ap, kb, b):
                for rc in range(nrc):
                    rcw = min(128, R - rc * 128)
                    nc.tensor.matmul(
                        po[rc][0:rcw, :],
                        lhsT=src_ap[0:kb, rc * 128 : rc * 128 + rcw],
                        rhs=haug[t][0:kb, b, j, :],
                        start=(b == 0),
                        stop=(b == nblk - 1),
                    )

            for g0 in range(0, nfull, G_BLOCKS):
                gw = min(G_BLOCKS, nfull - g0)
                atb = atpool.tile([128, G_BLOCKS, R], f16, name="atb", tag="atb")
                nc.sync.dma_start(
                    out=atb[:, 0:gw, :],
                    in_=at_dram[r][g0 * 128 : (g0 + gw) * 128, :].rearrange(
                        "(g p) r -> p g r", p=128
                    ),
                )
                for gi in range(gw):
                    mm_block(atb[:, gi, :], 128, g0 + gi)
            if tail:
                att = atpool.tile([128, R], f16, name="att", tag="att")
                nc.sync.dma_start(
                    out=att[0:tail, :],
                    in_=at_dram[r][nfull * 128 : nfull * 128 + tail, :],
                )
                mm_block(att, tail, nfull)

            # epilogue: acc += (A@H) * (1/rowsum); the elu "-1" is deferred
            for rc in range(nrc):
                rcw = min(128, R - rc * 128)
                rp = etmp.tile([128, 1], f32, name="rp", tag="rp")
                nc.vector.reciprocal(rp[0:rcw, :], po[rc][0:rcw, D : D + 1])
                nc.vector.scalar_tensor_tensor(
                    out=acc[dst][0:rcw, rc, :],
                    in0=po[rc][0:rcw, 0:D],
                    scalar=rp[0:rcw, :],
                    in1=acc[dst][0:rcw, rc, :],
                    op0=OP.mult,
                    op1=OP.add,
                )

        # ---- final: subtract elu count, l2-normalize (mean folds in) ----
        for t in TYPES:
            R = TYPE_N[t] // N_CORES
            nrc = _ceil_div(R, 128)
            n_elu = N_ELU_DEST[t]
            n_mean = N_MEAN[t]
            for rc in range(nrc):
                rcw = min(128, R - rc * 128)
                if n_elu:
                    am = outp.tile([128, D], f32, name="am", tag="am")
                    nc.vector.tensor_scalar(
                        am[0:rcw, :], acc[t][0:rcw, rc, :], float(n_elu), None,
                        OP.subtract,
                    )
                    amv = am[0:rcw, :]
                else:
                    amv = acc[t][0:rcw, rc, :]
                sq = etmp.tile([128, D], f32, name="sq", tag="sq")
                ss = etmp.tile([128, 1], f32, name="ss", tag="ss")
                nc.scalar.activation(
                    sq[0:rcw, :], amv, AF.Square, accum_out=ss[0:rcw, :]
                )
                nrm = etmp.tile([128, 1], f32, name="nrm", tag="nrm")
                nc.scalar.activation(nrm[0:rcw, :], ss[0:rcw, :], AF.Sqrt)
                nc.vector.tensor_scalar(
                    nrm[0:rcw, :], nrm[0:rcw, :], float(n_mean) * 1e-12, None, OP.max
                )
                rn = etmp.tile([128, 1], f32, name="rn", tag="rn")
                nc.vector.reciprocal(rn[0:rcw, :], nrm[0:rcw, :])
                ov = outp.tile([128, D], f32, name="ov", tag="ov")
                nc.vector.tensor_scalar(
                    ov[0:rcw, :], amv, rn[0:rcw, :], None, OP.mult
                )
                nc.scalar.dma_start(
                    out=out_dram[t][rc * 128 : rc * 128 + rcw, :], in_=ov[0:rcw, :]
                )

    _split_sync_waits(nc)
    return nc


def host_prep(inputs):
    """Shared (replicated) device inputs + per-core shards."""
    shared = {}
    for t in TYPES:
        X = inputs[FEAT_KEY[t]]
        S = TYPE_N[t]
        xa = np.empty((D + 1, S), np.float32)
        xa[0:D] = np.asarray(X, np.float32).T
        xa[D] = 1.0
        shared[f"xaugt_{t}"] = xa
        blocks = []
        for r in SRC_PACKS[t]:
            wb = np.empty((D + 1, D), np.float32)
            wb[0:D] = np.asarray(inputs["W_" + r], np.float32).T
            wb[D] = np.asarray(inputs["b_" + r], np.float32)
            blocks.append(wb)
        shared[f"wpack_{t}"] = np.ascontiguousarray(np.concatenate(blocks, axis=1))

    in_maps = []
    for c in range(N_CORES):
        m = dict(shared)
        for r in MAIN_ORDER:
            A = inputs["A_" + r]
            R = TYPE_N[REL_DEST[r]] // N_CORES
            m[f"at_{r}"] = A[c * R : (c + 1) * R, :].T.astype(np.float16)
        for t in TYPES:
            R = TYPE_N[t] // N_CORES
            F = inputs[FEAT_KEY[t]]
            m[f"featd_{t}"] = np.ascontiguousarray(
                np.asarray(F, np.float32)[c * R : (c + 1) * R, :]
            )
        in_maps.append(m)
    return in_maps


_NC_CACHE = None


def _get_nc():
    global _NC_CACHE
    if _NC_CACHE is None:
        _NC_CACHE = build_nc()
    return _NC_CACHE


def kernel(**inputs):
    from concourse.bass_utils import run_bass_kernel_spmd

    nc = _get_nc()
    in_maps = host_prep(inputs)

    kwargs = {}
    if os.environ.get("TRN_KERNEL_TRACE"):
        kwargs["trace"] = True
        tmpdir = os.environ.get("TRN_KERNEL_TRACE_DIR")
        if tmpdir:
            os.makedirs(tmpdir, exist_ok=True)
            kwargs["tmpdir"] = tmpdir

    res = run_bass_kernel_spmd(
        nc, in_maps, core_ids=list(range(N_CORES)), **kwargs
    )
    if os.environ.get("TRN_KERNEL_TRACE"):
        kernel.last_exec_time_ns = res.exec_time_ns

    outs = []
    for t in ["drug", "protein", "sideeffect", "disease"]:
        outs.append(
            np.concatenate([res.results[c][f"out_{t}"] for c in range(N_CORES)], axis=0)
        )
    return tuple(outs)
